# revision 38
# baseline (speedup 1.0000x reference)
"""Trainium2 Bass kernel for the CCN2 GNN message-passing module.

Shapes (hardcoded): B=16, n_nodes=1024, D=128, N=1025 (depot prepended).
Data-parallel over batch: 8 cores x 2 samples each.

Math per sample (reference semantics):
  A    = (dist(i,j) <= 0.055)            [N,N] 0/1, symmetric (self-loops incl.)
  B2   = ((A@A) > 0)                     [N,N] 0/1, symmetric
  C    = B2 * (B2@A)                     integer-valued
  fv0  = relu(x@W0 + b0)                 x = [loc_x, loc_y, deadline]
  fv1  = A @ fv0
  fv2  = C @ fv1
  out  = relu(fv2@W2 + b2), plus out.mean(axis=1)

Device mapping notes:
 - N padded to NP=1152 with far-away pad nodes (only self-adjacent).
 - A built exactly like the reference: s = fl(fl(dx^2)+fl(dy^2)) compared
   against fl(0.055^2) (verified boundary-flip-free vs sqrt comparison).
 - A/B2 hold 0/1 -> fp8(DoubleRow)/bf16 matmuls are EXACT (products exact,
   accumulation in fp32 PSUM). S/T contractions use fp8e4 DoubleRow over
   four K=256 superblocks plus one bf16 K=128 tail block.
 - fv0/fv1 are split into bf16 (hi, lo) pairs; A@(h+l) via PSUM
   accumulation gives ~2^-17 relative error.
 - C is used only transposed: CT = B2 .* (A@B2) = C^T, and
   fv2T = fv1^T @ C^T via matmul(lhsT=fv1, rhs=CT); the final
   out = fv2@W2 + b2 uses lhsT=fv2T directly. No transposes anywhere.
 - b0 is folded into the fv0 matmul via a constant ones row (K=4).
"""

import numpy as np

NP = 1152          # padded node count (9 x 128)
NT = NP // 128     # 128-row partition tiles
NSB = 4            # fp8 DoubleRow superblocks (K=256 each); tail block is bf16
D = 128
N = 1025
B_TOTAL = 16
N_CORES = 8
SPC = B_TOTAL // N_CORES   # samples per core
CHUNKS = [(0, 512), (512, 512), (1024, 16)]  # cols >1024 are padding; node 1024 is the only real one past 2x512
THRESH = np.float32(0.055)
T2 = float(THRESH * THRESH)

_CACHE = {}


def _build_nc():
    import concourse.tile as tile
    from concourse import bacc, mybir

    f32 = mybir.dt.float32
    bf16 = mybir.dt.bfloat16
    fp8 = mybir.dt.float8e4
    AF = mybir.ActivationFunctionType
    OP = mybir.AluOpType
    DR = mybir.MatmulPerfMode.DoubleRow

    nc = bacc.Bacc(
        "TRN2", target_bir_lowering=False, debug=False, num_devices=N_CORES
    )
    xq = nc.dram_tensor("xq", [SPC, 4, NP], f32, kind="ExternalInput").ap()
    ncol = nc.dram_tensor("ncol", [SPC, 128, 2 * NT], f32, kind="ExternalInput").ap()
    w04 = nc.dram_tensor("w04", [4, D], f32, kind="ExternalInput").ap()
    w2 = nc.dram_tensor("w2", [D, D], f32, kind="ExternalInput").ap()
    b2r = nc.dram_tensor("b2r", [1, D], f32, kind="ExternalInput").ap()
    out = nc.dram_tensor("out", [SPC, NP, D], f32, kind="ExternalOutput").ap()

    with tile.TileContext(nc) as tc:
        with (
            tc.tile_pool(name="const", bufs=1) as pc,
            tc.tile_pool(name="inp", bufs=2) as pin,
            tc.tile_pool(name="bc", bufs=2) as pbc,
            tc.tile_pool(name="sq", bufs=3) as psq,
            tc.tile_pool(name="af8pool", bufs=2) as pa8,
            tc.tile_pool(name="b2pool", bufs=2) as pb2,
            tc.tile_pool(name="ctpool", bufs=2) as pct,
            tc.tile_pool(name="fvpool", bufs=2) as pfv,
            tc.tile_pool(name="tmp", bufs=3) as ptmp,
            tc.tile_pool(name="f2t", bufs=2) as pf2t,
            tc.tile_pool(name="psbig", bufs=4, space="PSUM") as psbig,
            tc.tile_pool(name="psfv", bufs=2, space="PSUM") as psfv,
            tc.tile_pool(name="pso", bufs=2, space="PSUM") as pso,
        ):
            # ---- per-sample phase builders; explicitly interleaved -----
            def load_phase(b):
                st = {}
                # big broadcast DMAs first, on otherwise-idle engine queues,
                # so the A-build squares can start ASAP
                xj = pbc.tile([128, NP], f32, tag="xj", name="xj")
                nc.sync.dma_start(xj[:], xq[b, 0, :].partition_broadcast(128))
                yj = pbc.tile([128, NP], f32, tag="yj", name="yj")
                nc.scalar.dma_start(yj[:], xq[b, 1, :].partition_broadcast(128))
                xt4 = pin.tile([4, NP], f32, tag="xt4", name="xt4")
                nc.sync.dma_start(xt4[:], xq[b])
                ncols = pin.tile([128, 2 * NT], f32, tag="ncols", name="ncols")
                nc.sync.dma_start(ncols[:], ncol[b])
                st.update(xt4=xt4, ncols=ncols, xj=xj, yj=yj)
                return st

            def a_phase(st, b):
                xj, yj = st["xj"], st["yj"]
                Af8 = [
                    pa8.tile([128, 2, NP], fp8, tag=f"Af8{s}", name=f"Af8_{s}")
                    for s in range(NSB)
                ]
                a8 = pa8.tile([128, NP], fp8, tag="A8tail", name="a8")

                def a_row(kt):
                    return Af8[kt // 2][:, kt % 2, :] if kt < 2 * NSB else a8[:]

                ncols = st["ncols"]
                for mt in range(NT):
                    dx2 = psq.tile([128, NP], f32, tag="dx2", name="dx2")
                    nc.scalar.activation(
                        dx2[:], xj[:], AF.Square, bias=ncols[:, mt : mt + 1]
                    )
                    dy2 = psq.tile([128, NP], f32, tag="dy2", name="dy2")
                    nc.scalar.activation(
                        dy2[:], yj[:], AF.Square, bias=ncols[:, NT + mt : NT + mt + 1]
                    )
                    # d2 = dx2 + dy2 in place. Sample 0 builds at kernel start
                    # (GPSIMD dispatch is laggy there) -> DVE; sample 1 builds
                    # while DVE is busy with sample 0's S phase -> GPSIMD.
                    if b == 0:
                        nc.vector.tensor_add(dx2[:], dx2[:], dy2[:])
                    else:
                        nc.gpsimd.tensor_add(dx2[:], dx2[:], dy2[:])
                    nc.vector.tensor_scalar(a_row(mt), dx2[:], T2, None, OP.is_le)
                st.update(Af8=Af8, a8=a8, a_row=a_row)

            def fv0_phase(st):
                f0hl = []
                xt4 = st["xt4"]
                for mt in range(NT):
                    ms = slice(mt * 128, (mt + 1) * 128)
                    ps0 = psfv.tile([128, D], f32, tag="psf", name="ps0")
                    nc.tensor.matmul(ps0[:], xt4[:, ms], w0t[:], start=True, stop=True)
                    f0f = ptmp.tile([128, D], f32, tag="f0f", name="f0f")
                    nc.scalar.activation(f0f[:], ps0[:], AF.Relu)
                    hl = pfv.tile([128, 2 * D], bf16, tag=f"f0hl{mt}", name="f0hl")
                    nc.vector.tensor_copy(hl[:, 0:D], f0f[:])
                    nc.vector.tensor_tensor(
                        hl[:, D : 2 * D], f0f[:], hl[:, 0:D], OP.subtract
                    )
                    f0hl.append(hl)
                st["f0hl"] = f0hl

            def s_phase(st, b):
                Af8, a8 = st["Af8"], st["a8"]
                B2f8 = [
                    pb2.tile([128, 2, NP], fp8, tag=f"B2f8{s}", name=f"B2f8_{s}")
                    for s in range(NSB)
                ]
                b28 = pb2.tile([128, NP], fp8, tag="B28", name="b28")
                for mt in range(NT):
                    ms = slice(mt * 128, (mt + 1) * 128)
                    for ci, (c0, cw) in enumerate(CHUNKS):
                        ps = psbig.tile([128, 512], f32, tag="psbig", name="psS")
                        for s in range(NSB):
                            nc.tensor.matmul(
                                ps[:, 0:cw],
                                Af8[s][:, :, ms],
                                Af8[s][:, :, c0 : c0 + cw],
                                start=(s == 0),
                                stop=False,
                                perf_mode=DR,
                            )
                        nc.tensor.matmul(
                            ps[:, 0:cw],
                            a8[:, ms],
                            a8[:, c0 : c0 + cw],
                            start=False,
                            stop=True,
                        )
                        # B2 = (S > 0) for integer counts. During sample 0's S
                        # phase ACT is saturated by sample 1's squares -> DVE;
                        # during sample 1's S phase ACT is free -> ACT Sign.
                        dst = (
                            B2f8[mt // 2][:, mt % 2, c0 : c0 + cw]
                            if mt < 2 * NSB
                            else b28[:, c0 : c0 + cw]
                        )
                        if b == 0:
                            nc.vector.tensor_scalar(
                                dst, ps[:, 0:cw], 0.5, None, OP.is_ge
                            )
                        else:
                            nc.scalar.activation(dst, ps[:, 0:cw], AF.Sign)
                st.update(B2f8=B2f8, b28=b28)

            def t_phase(st):
                Af8, a8, B2f8, b28 = st["Af8"], st["a8"], st["B2f8"], st["b28"]
                CT = []
                for mt in range(NT):
                    ms = slice(mt * 128, (mt + 1) * 128)
                    ct_t = pct.tile([128, NP], bf16, tag=f"CT{mt}", name="ct")
                    if mt < 2 * NSB:
                        b2row = B2f8[mt // 2][:, mt % 2, :]
                    else:
                        b2row = b28[:]
                    for ci, (c0, cw) in enumerate(CHUNKS):
                        ps = psbig.tile([128, 512], f32, tag="psbig", name="psT")
                        for s in range(NSB):
                            nc.tensor.matmul(
                                ps[:, 0:cw],
                                Af8[s][:, :, ms],
                                B2f8[s][:, :, c0 : c0 + cw],
                                start=(s == 0),
                                stop=False,
                                perf_mode=DR,
                            )
                        nc.tensor.matmul(
                            ps[:, 0:cw],
                            a8[:, ms],
                            b28[:, c0 : c0 + cw],
                            start=False,
                            stop=True,
                        )
                        nc.vector.tensor_tensor(
                            ct_t[:, c0 : c0 + cw],
                            b2row[:, c0 : c0 + cw],
                            ps[:, 0:cw],
                            OP.mult,
                        )
                    CT.append(ct_t)
                st["CT"] = CT

            def fv1_phase(st):
                a_row, f0hl = st["a_row"], st["f0hl"]
                f1hl = []
                for mt in range(NT):
                    ms = slice(mt * 128, (mt + 1) * 128)
                    ps1 = psfv.tile([128, 2 * D], f32, tag="psf", name="ps1")
                    for kt in range(NT):
                        nc.tensor.matmul(
                            ps1[:],
                            a_row(kt)[:, ms],
                            f0hl[kt][:],
                            start=(kt == 0),
                            stop=(kt == NT - 1),
                        )
                    # fv1 = psum[:, 0:D] + psum[:, D:2D] via strided reduce
                    f1f = ptmp.tile([128, D], f32, tag="f1f", name="f1f")
                    nc.vector.tensor_reduce(
                        f1f[:],
                        ps1[:].rearrange("p (d j) -> p j d", d=2),
                        mybir.AxisListType.X,
                        OP.add,
                    )
                    hl = pfv.tile([128, 2 * D], bf16, tag=f"f1hl{mt}", name="f1hl")
                    nc.vector.tensor_copy(hl[:, 0:D], f1f[:])
                    nc.vector.tensor_tensor(
                        hl[:, D : 2 * D], f1f[:], hl[:, 0:D], OP.subtract
                    )
                    f1hl.append(hl)
                st["f1hl"] = f1hl

            def out_tile(st, b, mt):
                f2t = st["f2t"]
                ms = slice(mt * 128, (mt + 1) * 128)
                pso_t = pso.tile([128, D], f32, tag="pso", name="pso_t")
                nc.tensor.matmul(pso_t[:], f2t[:, ms], w2t[:], start=True, stop=False)
                nc.tensor.matmul(pso_t[:], ones1[:], b2t[:], start=False, stop=True)
                osb = ptmp.tile([128, D], f32, tag="osb", name="osb")
                nc.scalar.activation(osb[:], pso_t[:], AF.Relu)
                nc.sync.dma_start(out[b, ms, :], osb[:])

            # out row-tiles fully covered once chunk ci of f2t is written
            OUT_MTS = [range(0, 4), range(4, 8), range(8, NT)]

            def f2t_phase(st, b):
                f1hl, CT = st["f1hl"], st["CT"]
                f2t = pf2t.tile([128, NP], f32, tag="f2t", name="f2t")
                st["f2t"] = f2t
                for ci, (c0, cw) in enumerate(CHUNKS):
                    ps = psbig.tile([128, 512], f32, tag="psbig", name="psF")
                    for kt in range(NT):
                        for part in range(2):
                            nc.tensor.matmul(
                                ps[:, 0:cw],
                                f1hl[kt][:, part * D : (part + 1) * D],
                                CT[kt][:, c0 : c0 + cw],
                                start=(kt == 0 and part == 0),
                                stop=(kt == NT - 1 and part == 1),
                            )
                    nc.vector.tensor_copy(f2t[:, c0 : c0 + cw], ps[:, 0:cw])
                    for mt in OUT_MTS[ci]:
                        out_tile(st, b, mt)

            st0 = load_phase(0)
            w0t = pc.tile([4, D], f32)
            nc.sync.dma_start(w0t[:], w04[:])
            w2t = pc.tile([D, D], f32)
            nc.sync.dma_start(w2t[:], w2[:])
            b2t = pc.tile([1, D], f32)
            nc.sync.dma_start(b2t[:], b2r[:])
            ones1 = pc.tile([1, D], f32)
            nc.vector.memset(ones1[:], 1.0)
            st1 = load_phase(1)
            a_phase(st0, 0)
            fv0_phase(st0)
            fv0_phase(st1)
            a_phase(st1, 1)
            s_phase(st0, 0)
            fv1_phase(st0)
            s_phase(st1, 1)
            fv1_phase(st1)
            t_phase(st0)
            t_phase(st1)
            f2t_phase(st0, 0)
            f2t_phase(st1, 1)

    nc.compile()
    return nc


def _get_nc():
    if "nc" not in _CACHE:
        _CACHE["nc"] = _build_nc()
    return _CACHE["nc"]


def _prep_in_maps(loc, deadline, depot, W0, b0, W2, b2):
    B = loc.shape[0]
    x = np.zeros((B, NP), np.float32)
    y = np.zeros((B, NP), np.float32)
    t = np.zeros((B, NP), np.float32)
    o = np.ones((B, NP), np.float32)
    x[:, 0] = depot[:, 0]
    y[:, 0] = depot[:, 1]
    x[:, 1:N] = loc[:, :, 0]
    y[:, 1:N] = loc[:, :, 1]
    t[:, 1:N] = deadline
    # pad nodes: far away and pairwise > thresh apart
    x[:, N:] = 3.0 + np.arange(NP - N, dtype=np.float32)
    y[:, N:] = 0.5
    xq = np.stack([x, y, t, o], axis=1).astype(np.float32)       # [B, 4, NP]
    ncol = np.ascontiguousarray(
        np.stack([-x, -y], axis=1)
        .reshape(B, 2 * NT, 128)
        .transpose(0, 2, 1)
        .astype(np.float32)
    )  # [B, 128, 2*NT]: partition-major so the DMA is contiguous
    w04 = np.concatenate(
        [np.asarray(W0, np.float32), np.asarray(b0, np.float32).reshape(1, D)], axis=0
    )
    W2 = np.asarray(W2, np.float32)
    b2 = np.asarray(b2, np.float32).reshape(1, D)
    in_maps = []
    for c in range(N_CORES):
        s = slice(c * SPC, (c + 1) * SPC)
        in_maps.append(
            {
                "xq": np.ascontiguousarray(xq[s]),
                "ncol": np.ascontiguousarray(ncol[s]),
                "w04": w04,
                "w2": W2,
                "b2r": b2,
            }
        )
    return in_maps


def _run_device(in_maps):
    import time

    from concourse.bass_utils import run_bass_kernel_spmd

    nc = _get_nc()
    last_err = None
    for attempt in range(3):
        try:
            res = run_bass_kernel_spmd(nc, in_maps, list(range(N_CORES)))
            return res.results
        except Exception as e:  # transient device errors (e.g. NRT_EXEC_UNIT_*)
            last_err = e
            time.sleep(2.0 * (attempt + 1))
    raise last_err


def kernel(loc, deadline, depot, W0, b0, W2, b2):
    loc = np.asarray(loc, np.float32)
    deadline = np.asarray(deadline, np.float32)
    depot = np.asarray(depot, np.float32)
    in_maps = _prep_in_maps(loc, deadline, depot, W0, b0, W2, b2)
    results = _run_device(in_maps)
    out = np.empty((B_TOTAL, N, D), np.float32)
    for c in range(N_CORES):
        out[c * SPC : (c + 1) * SPC] = results[c]["out"][:, :N, :]
    mean = out.mean(axis=1, dtype=np.float64).astype(np.float32)
    return out, mean


# revision 39
# speedup vs baseline: 1.0056x; 1.0056x over previous
"""Trainium2 Bass kernel for the CCN2 GNN message-passing module.

Shapes (hardcoded): B=16, n_nodes=1024, D=128, N=1025 (depot prepended).
Data-parallel over batch: 8 cores x 2 samples each.

Math per sample (reference semantics):
  A    = (dist(i,j) <= 0.055)            [N,N] 0/1, symmetric (self-loops incl.)
  B2   = ((A@A) > 0)                     [N,N] 0/1, symmetric
  C    = B2 * (B2@A)                     integer-valued
  fv0  = relu(x@W0 + b0)                 x = [loc_x, loc_y, deadline]
  fv1  = A @ fv0
  fv2  = C @ fv1
  out  = relu(fv2@W2 + b2), plus out.mean(axis=1)

Device mapping notes:
 - N padded to NP=1152 with far-away pad nodes (only self-adjacent).
 - A built exactly like the reference: s = fl(fl(dx^2)+fl(dy^2)) compared
   against fl(0.055^2) (verified boundary-flip-free vs sqrt comparison).
 - A/B2 hold 0/1 -> fp8(DoubleRow)/bf16 matmuls are EXACT (products exact,
   accumulation in fp32 PSUM). S/T contractions use fp8e4 DoubleRow over
   four K=256 superblocks plus one bf16 K=128 tail block.
 - fv0/fv1 are split into bf16 (hi, lo) pairs; A@(h+l) via PSUM
   accumulation gives ~2^-17 relative error.
 - C is used only transposed: CT = B2 .* (A@B2) = C^T, and
   fv2T = fv1^T @ C^T via matmul(lhsT=fv1, rhs=CT); the final
   out = fv2@W2 + b2 uses lhsT=fv2T directly. No transposes anywhere.
 - b0 is folded into the fv0 matmul via a constant ones row (K=4).
"""

import numpy as np

NP = 1152          # padded node count (9 x 128)
NT = NP // 128     # 128-row partition tiles
NSB = 4            # fp8 DoubleRow superblocks (K=256 each); tail block is bf16
D = 128
N = 1025
B_TOTAL = 16
N_CORES = 8
SPC = B_TOTAL // N_CORES   # samples per core
CHUNKS = [(0, 512), (512, 512), (1024, 16)]  # cols >1024 are padding; node 1024 is the only real one past 2x512
THRESH = np.float32(0.055)
T2 = float(THRESH * THRESH)

_CACHE = {}


def _build_nc():
    import concourse.tile as tile
    from concourse import bacc, mybir

    f32 = mybir.dt.float32
    bf16 = mybir.dt.bfloat16
    fp8 = mybir.dt.float8e4
    AF = mybir.ActivationFunctionType
    OP = mybir.AluOpType
    DR = mybir.MatmulPerfMode.DoubleRow

    nc = bacc.Bacc(
        "TRN2", target_bir_lowering=False, debug=False, num_devices=N_CORES
    )
    xq = nc.dram_tensor("xq", [SPC, 4, NP], f32, kind="ExternalInput").ap()
    ncol = nc.dram_tensor("ncol", [SPC, 128, 2 * NT], f32, kind="ExternalInput").ap()
    w04 = nc.dram_tensor("w04", [4, D], f32, kind="ExternalInput").ap()
    w2 = nc.dram_tensor("w2", [D, D], f32, kind="ExternalInput").ap()
    b2r = nc.dram_tensor("b2r", [1, D], f32, kind="ExternalInput").ap()
    out = nc.dram_tensor("out", [SPC, NP, D], f32, kind="ExternalOutput").ap()

    with tile.TileContext(nc) as tc:
        with (
            tc.tile_pool(name="const", bufs=1) as pc,
            tc.tile_pool(name="inp", bufs=2) as pin,
            tc.tile_pool(name="bc", bufs=2) as pbc,
            tc.tile_pool(name="sq", bufs=4) as psq,
            tc.tile_pool(name="af8pool", bufs=2) as pa8,
            tc.tile_pool(name="b2pool", bufs=2) as pb2,
            tc.tile_pool(name="ctpool", bufs=2) as pct,
            tc.tile_pool(name="fvpool", bufs=2) as pfv,
            tc.tile_pool(name="tmp", bufs=3) as ptmp,
            tc.tile_pool(name="f2t", bufs=2) as pf2t,
            tc.tile_pool(name="psbig", bufs=4, space="PSUM") as psbig,
            tc.tile_pool(name="psfv", bufs=2, space="PSUM") as psfv,
            tc.tile_pool(name="pso", bufs=2, space="PSUM") as pso,
        ):
            # ---- per-sample phase builders; explicitly interleaved -----
            def load_phase(b):
                st = {}
                # big broadcast DMAs first, on otherwise-idle engine queues,
                # so the A-build squares can start ASAP
                xj = pbc.tile([128, NP], f32, tag="xj", name="xj")
                nc.sync.dma_start(xj[:], xq[b, 0, :].partition_broadcast(128))
                yj = pbc.tile([128, NP], f32, tag="yj", name="yj")
                nc.scalar.dma_start(yj[:], xq[b, 1, :].partition_broadcast(128))
                xt4 = pin.tile([4, NP], f32, tag="xt4", name="xt4")
                nc.sync.dma_start(xt4[:], xq[b])
                ncols = pin.tile([128, 2 * NT], f32, tag="ncols", name="ncols")
                nc.sync.dma_start(ncols[:], ncol[b])
                st.update(xt4=xt4, ncols=ncols, xj=xj, yj=yj)
                return st

            def a_phase(st, b):
                xj, yj = st["xj"], st["yj"]
                Af8 = [
                    pa8.tile([128, 2, NP], fp8, tag=f"Af8{s}", name=f"Af8_{s}")
                    for s in range(NSB)
                ]
                a8 = pa8.tile([128, NP], fp8, tag="A8tail", name="a8")

                def a_row(kt):
                    return Af8[kt // 2][:, kt % 2, :] if kt < 2 * NSB else a8[:]

                ncols = st["ncols"]
                for mt in range(NT):
                    dx2 = psq.tile([128, NP], f32, tag="dx2", name="dx2")
                    nc.scalar.activation(
                        dx2[:], xj[:], AF.Square, bias=ncols[:, mt : mt + 1]
                    )
                    dy2 = psq.tile([128, NP], f32, tag="dy2", name="dy2")
                    nc.scalar.activation(
                        dy2[:], yj[:], AF.Square, bias=ncols[:, NT + mt : NT + mt + 1]
                    )
                    # d2 = dx2 + dy2 in place. Sample 0 builds at kernel start
                    # (GPSIMD dispatch is laggy there) -> DVE; sample 1 builds
                    # while DVE is busy with sample 0's S phase -> GPSIMD.
                    if b == 0:
                        nc.vector.tensor_add(dx2[:], dx2[:], dy2[:])
                    else:
                        nc.gpsimd.tensor_add(dx2[:], dx2[:], dy2[:])
                    nc.vector.tensor_scalar(a_row(mt), dx2[:], T2, None, OP.is_le)
                st.update(Af8=Af8, a8=a8, a_row=a_row)

            def fv0_phase(st):
                f0hl = []
                xt4 = st["xt4"]
                for mt in range(NT):
                    ms = slice(mt * 128, (mt + 1) * 128)
                    ps0 = psfv.tile([128, D], f32, tag="psf", name="ps0")
                    nc.tensor.matmul(ps0[:], xt4[:, ms], w0t[:], start=True, stop=True)
                    f0f = ptmp.tile([128, D], f32, tag="f0f", name="f0f")
                    nc.scalar.activation(f0f[:], ps0[:], AF.Relu)
                    hl = pfv.tile([128, 2 * D], bf16, tag=f"f0hl{mt}", name="f0hl")
                    nc.vector.tensor_copy(hl[:, 0:D], f0f[:])
                    nc.vector.tensor_tensor(
                        hl[:, D : 2 * D], f0f[:], hl[:, 0:D], OP.subtract
                    )
                    f0hl.append(hl)
                st["f0hl"] = f0hl

            def s_phase(st, b):
                Af8, a8 = st["Af8"], st["a8"]
                B2f8 = [
                    pb2.tile([128, 2, NP], fp8, tag=f"B2f8{s}", name=f"B2f8_{s}")
                    for s in range(NSB)
                ]
                b28 = pb2.tile([128, NP], fp8, tag="B28", name="b28")
                for mt in range(NT):
                    ms = slice(mt * 128, (mt + 1) * 128)
                    for ci, (c0, cw) in enumerate(CHUNKS):
                        ps = psbig.tile([128, 512], f32, tag="psbig", name="psS")
                        for s in range(NSB):
                            nc.tensor.matmul(
                                ps[:, 0:cw],
                                Af8[s][:, :, ms],
                                Af8[s][:, :, c0 : c0 + cw],
                                start=(s == 0),
                                stop=False,
                                perf_mode=DR,
                            )
                        nc.tensor.matmul(
                            ps[:, 0:cw],
                            a8[:, ms],
                            a8[:, c0 : c0 + cw],
                            start=False,
                            stop=True,
                        )
                        # B2 = (S > 0) for integer counts. During sample 0's S
                        # phase ACT is saturated by sample 1's squares -> DVE;
                        # during sample 1's S phase ACT is free -> ACT Sign.
                        dst = (
                            B2f8[mt // 2][:, mt % 2, c0 : c0 + cw]
                            if mt < 2 * NSB
                            else b28[:, c0 : c0 + cw]
                        )
                        if b == 0:
                            nc.vector.tensor_scalar(
                                dst, ps[:, 0:cw], 0.5, None, OP.is_ge
                            )
                        else:
                            nc.scalar.activation(dst, ps[:, 0:cw], AF.Sign)
                st.update(B2f8=B2f8, b28=b28)

            def t_phase(st):
                Af8, a8, B2f8, b28 = st["Af8"], st["a8"], st["B2f8"], st["b28"]
                CT = []
                for mt in range(NT):
                    ms = slice(mt * 128, (mt + 1) * 128)
                    ct_t = pct.tile([128, NP], bf16, tag=f"CT{mt}", name="ct")
                    if mt < 2 * NSB:
                        b2row = B2f8[mt // 2][:, mt % 2, :]
                    else:
                        b2row = b28[:]
                    for ci, (c0, cw) in enumerate(CHUNKS):
                        ps = psbig.tile([128, 512], f32, tag="psbig", name="psT")
                        for s in range(NSB):
                            nc.tensor.matmul(
                                ps[:, 0:cw],
                                Af8[s][:, :, ms],
                                B2f8[s][:, :, c0 : c0 + cw],
                                start=(s == 0),
                                stop=False,
                                perf_mode=DR,
                            )
                        nc.tensor.matmul(
                            ps[:, 0:cw],
                            a8[:, ms],
                            b28[:, c0 : c0 + cw],
                            start=False,
                            stop=True,
                        )
                        nc.vector.tensor_tensor(
                            ct_t[:, c0 : c0 + cw],
                            b2row[:, c0 : c0 + cw],
                            ps[:, 0:cw],
                            OP.mult,
                        )
                    CT.append(ct_t)
                st["CT"] = CT

            def fv1_phase(st):
                a_row, f0hl = st["a_row"], st["f0hl"]
                f1hl = []
                for mt in range(NT):
                    ms = slice(mt * 128, (mt + 1) * 128)
                    ps1 = psfv.tile([128, 2 * D], f32, tag="psf", name="ps1")
                    for kt in range(NT):
                        nc.tensor.matmul(
                            ps1[:],
                            a_row(kt)[:, ms],
                            f0hl[kt][:],
                            start=(kt == 0),
                            stop=(kt == NT - 1),
                        )
                    # fv1 = psum[:, 0:D] + psum[:, D:2D] via strided reduce
                    f1f = ptmp.tile([128, D], f32, tag="f1f", name="f1f")
                    nc.vector.tensor_reduce(
                        f1f[:],
                        ps1[:].rearrange("p (d j) -> p j d", d=2),
                        mybir.AxisListType.X,
                        OP.add,
                    )
                    hl = pfv.tile([128, 2 * D], bf16, tag=f"f1hl{mt}", name="f1hl")
                    nc.vector.tensor_copy(hl[:, 0:D], f1f[:])
                    nc.vector.tensor_tensor(
                        hl[:, D : 2 * D], f1f[:], hl[:, 0:D], OP.subtract
                    )
                    f1hl.append(hl)
                st["f1hl"] = f1hl

            def out_tile(st, b, mt):
                f2t = st["f2t"]
                ms = slice(mt * 128, (mt + 1) * 128)
                pso_t = pso.tile([128, D], f32, tag="pso", name="pso_t")
                nc.tensor.matmul(pso_t[:], f2t[:, ms], w2t[:], start=True, stop=False)
                nc.tensor.matmul(pso_t[:], ones1[:], b2t[:], start=False, stop=True)
                osb = ptmp.tile([128, D], f32, tag="osb", name="osb")
                nc.scalar.activation(osb[:], pso_t[:], AF.Relu)
                nc.sync.dma_start(out[b, ms, :], osb[:])

            # out row-tiles fully covered once chunk ci of f2t is written
            OUT_MTS = [range(0, 4), range(4, 8), range(8, NT)]

            def f2t_phase(st, b):
                f1hl, CT = st["f1hl"], st["CT"]
                f2t = pf2t.tile([128, NP], f32, tag="f2t", name="f2t")
                st["f2t"] = f2t
                for ci, (c0, cw) in enumerate(CHUNKS):
                    ps = psbig.tile([128, 512], f32, tag="psbig", name="psF")
                    for kt in range(NT):
                        for part in range(2):
                            nc.tensor.matmul(
                                ps[:, 0:cw],
                                f1hl[kt][:, part * D : (part + 1) * D],
                                CT[kt][:, c0 : c0 + cw],
                                start=(kt == 0 and part == 0),
                                stop=(kt == NT - 1 and part == 1),
                            )
                    nc.vector.tensor_copy(f2t[:, c0 : c0 + cw], ps[:, 0:cw])
                    for mt in OUT_MTS[ci]:
                        out_tile(st, b, mt)

            st0 = load_phase(0)
            w0t = pc.tile([4, D], f32)
            nc.sync.dma_start(w0t[:], w04[:])
            w2t = pc.tile([D, D], f32)
            nc.sync.dma_start(w2t[:], w2[:])
            b2t = pc.tile([1, D], f32)
            nc.sync.dma_start(b2t[:], b2r[:])
            ones1 = pc.tile([1, D], f32)
            nc.vector.memset(ones1[:], 1.0)
            st1 = load_phase(1)
            a_phase(st0, 0)
            fv0_phase(st0)
            fv0_phase(st1)
            a_phase(st1, 1)
            s_phase(st0, 0)
            fv1_phase(st0)
            s_phase(st1, 1)
            fv1_phase(st1)
            t_phase(st0)
            t_phase(st1)
            f2t_phase(st0, 0)
            f2t_phase(st1, 1)

    nc.compile()
    return nc


def _get_nc():
    if "nc" not in _CACHE:
        _CACHE["nc"] = _build_nc()
    return _CACHE["nc"]


def _prep_in_maps(loc, deadline, depot, W0, b0, W2, b2):
    B = loc.shape[0]
    x = np.zeros((B, NP), np.float32)
    y = np.zeros((B, NP), np.float32)
    t = np.zeros((B, NP), np.float32)
    o = np.ones((B, NP), np.float32)
    x[:, 0] = depot[:, 0]
    y[:, 0] = depot[:, 1]
    x[:, 1:N] = loc[:, :, 0]
    y[:, 1:N] = loc[:, :, 1]
    t[:, 1:N] = deadline
    # pad nodes: far away and pairwise > thresh apart
    x[:, N:] = 3.0 + np.arange(NP - N, dtype=np.float32)
    y[:, N:] = 0.5
    xq = np.stack([x, y, t, o], axis=1).astype(np.float32)       # [B, 4, NP]
    ncol = np.ascontiguousarray(
        np.stack([-x, -y], axis=1)
        .reshape(B, 2 * NT, 128)
        .transpose(0, 2, 1)
        .astype(np.float32)
    )  # [B, 128, 2*NT]: partition-major so the DMA is contiguous
    w04 = np.concatenate(
        [np.asarray(W0, np.float32), np.asarray(b0, np.float32).reshape(1, D)], axis=0
    )
    W2 = np.asarray(W2, np.float32)
    b2 = np.asarray(b2, np.float32).reshape(1, D)
    in_maps = []
    for c in range(N_CORES):
        s = slice(c * SPC, (c + 1) * SPC)
        in_maps.append(
            {
                "xq": np.ascontiguousarray(xq[s]),
                "ncol": np.ascontiguousarray(ncol[s]),
                "w04": w04,
                "w2": W2,
                "b2r": b2,
            }
        )
    return in_maps


def _run_device(in_maps):
    import time

    from concourse.bass_utils import run_bass_kernel_spmd

    nc = _get_nc()
    last_err = None
    for attempt in range(3):
        try:
            res = run_bass_kernel_spmd(nc, in_maps, list(range(N_CORES)))
            return res.results
        except Exception as e:  # transient device errors (e.g. NRT_EXEC_UNIT_*)
            last_err = e
            time.sleep(2.0 * (attempt + 1))
    raise last_err


def kernel(loc, deadline, depot, W0, b0, W2, b2):
    loc = np.asarray(loc, np.float32)
    deadline = np.asarray(deadline, np.float32)
    depot = np.asarray(depot, np.float32)
    in_maps = _prep_in_maps(loc, deadline, depot, W0, b0, W2, b2)
    results = _run_device(in_maps)
    out = np.empty((B_TOTAL, N, D), np.float32)
    for c in range(N_CORES):
        out[c * SPC : (c + 1) * SPC] = results[c]["out"][:, :N, :]
    mean = out.mean(axis=1, dtype=np.float64).astype(np.float32)
    return out, mean


# revision 42
# speedup vs baseline: 1.0359x; 1.0301x over previous
"""Trainium2 Bass kernel for the CCN2 GNN message-passing module.

Shapes (hardcoded): B=16, n_nodes=1024, D=128, N=1025 (depot prepended).
Data-parallel over batch: 8 cores x 2 samples each.

Math per sample (reference semantics):
  A    = (dist(i,j) <= 0.055)            [N,N] 0/1, symmetric (self-loops incl.)
  B2   = ((A@A) > 0)                     [N,N] 0/1, symmetric
  C    = B2 * (B2@A)                     integer-valued
  fv0  = relu(x@W0 + b0)                 x = [loc_x, loc_y, deadline]
  fv1  = A @ fv0
  fv2  = C @ fv1
  out  = relu(fv2@W2 + b2), plus out.mean(axis=1)

Device mapping notes:
 - N padded to NP=1152 with far-away pad nodes (only self-adjacent).
 - A built exactly like the reference: s = fl(fl(dx^2)+fl(dy^2)) compared
   against fl(0.055^2) (verified boundary-flip-free vs sqrt comparison).
 - A/B2 hold 0/1 -> fp8(DoubleRow)/bf16 matmuls are EXACT (products exact,
   accumulation in fp32 PSUM). S/T contractions use fp8e4 DoubleRow over
   four K=256 superblocks plus one bf16 K=128 tail block.
 - fv0/fv1 are split into bf16 (hi, lo) pairs; A@(h+l) via PSUM
   accumulation gives ~2^-17 relative error.
 - C is used only transposed: CT = B2 .* (A@B2) = C^T, and
   fv2T = fv1^T @ C^T via matmul(lhsT=fv1, rhs=CT); the final
   out = fv2@W2 + b2 uses lhsT=fv2T directly. No transposes anywhere.
 - b0 is folded into the fv0 matmul via a constant ones row (K=4).
"""

import numpy as np

NP = 1152          # padded node count (9 x 128)
NT = NP // 128     # 128-row partition tiles
NSB = 4            # fp8 DoubleRow superblocks (K=256 each); tail block is bf16
D = 128
N = 1025
B_TOTAL = 16
N_CORES = 8
SPC = B_TOTAL // N_CORES   # samples per core
CHUNKS = [(0, 512), (512, 512), (1024, 16)]  # cols >1024 are padding; node 1024 is the only real one past 2x512
THRESH = np.float32(0.055)
T2 = float(THRESH * THRESH)

_CACHE = {}


def _build_nc():
    import concourse.tile as tile
    from concourse import bacc, mybir

    f32 = mybir.dt.float32
    bf16 = mybir.dt.bfloat16
    fp8 = mybir.dt.float8e4
    AF = mybir.ActivationFunctionType
    OP = mybir.AluOpType
    DR = mybir.MatmulPerfMode.DoubleRow

    nc = bacc.Bacc(
        "TRN2", target_bir_lowering=False, debug=False, num_devices=N_CORES
    )
    xq = nc.dram_tensor("xq", [SPC, 4, NP], f32, kind="ExternalInput").ap()
    ncol = nc.dram_tensor("ncol", [SPC, 128, 2 * NT], f32, kind="ExternalInput").ap()
    w04 = nc.dram_tensor("w04", [4, D], f32, kind="ExternalInput").ap()
    w2 = nc.dram_tensor("w2", [D, D], f32, kind="ExternalInput").ap()
    b2r = nc.dram_tensor("b2r", [1, D], f32, kind="ExternalInput").ap()
    out = nc.dram_tensor("out", [SPC, NP, D], f32, kind="ExternalOutput").ap()

    with tile.TileContext(nc) as tc:
        with (
            tc.tile_pool(name="const", bufs=1) as pc,
            tc.tile_pool(name="inp", bufs=2) as pin,
            tc.tile_pool(name="bc", bufs=2) as pbc,
            tc.tile_pool(name="sq", bufs=4) as psq,
            tc.tile_pool(name="af8pool", bufs=2) as pa8,
            tc.tile_pool(name="b2pool", bufs=2) as pb2,
            tc.tile_pool(name="ctpool", bufs=2) as pct,
            tc.tile_pool(name="fvpool", bufs=2) as pfv,
            tc.tile_pool(name="tmp", bufs=3) as ptmp,
            tc.tile_pool(name="f2t", bufs=2) as pf2t,
            tc.tile_pool(name="psbig", bufs=4, space="PSUM") as psbig,
            tc.tile_pool(name="psfv", bufs=2, space="PSUM") as psfv,
            tc.tile_pool(name="pso", bufs=2, space="PSUM") as pso,
        ):
            # ---- per-sample phase builders; explicitly interleaved -----
            def load_phase(b):
                st = {}
                # big broadcast DMAs first, on otherwise-idle engine queues,
                # so the A-build squares can start ASAP
                xj = pbc.tile([128, NP], f32, tag="xj", name="xj")
                nc.sync.dma_start(xj[:], xq[b, 0, :].partition_broadcast(128))
                yj = pbc.tile([128, NP], f32, tag="yj", name="yj")
                nc.scalar.dma_start(yj[:], xq[b, 1, :].partition_broadcast(128))
                xt4 = pin.tile([4, NP], f32, tag="xt4", name="xt4")
                nc.sync.dma_start(xt4[:], xq[b])
                ncols = pin.tile([128, 2 * NT], f32, tag="ncols", name="ncols")
                nc.sync.dma_start(ncols[:], ncol[b])
                st.update(xt4=xt4, ncols=ncols, xj=xj, yj=yj)
                return st

            def a_phase(st, b):
                xj, yj = st["xj"], st["yj"]
                Af8 = [
                    pa8.tile([128, 2, NP], fp8, tag=f"Af8{s}", name=f"Af8_{s}")
                    for s in range(NSB)
                ]
                a8 = pa8.tile([128, NP], fp8, tag="A8tail", name="a8")

                def a_row(kt):
                    return Af8[kt // 2][:, kt % 2, :] if kt < 2 * NSB else a8[:]

                ncols = st["ncols"]
                for mt in range(NT):
                    dx2 = psq.tile([128, NP], f32, tag="dx2", name="dx2")
                    nc.scalar.activation(
                        dx2[:], xj[:], AF.Square, bias=ncols[:, mt : mt + 1]
                    )
                    dy2 = psq.tile([128, NP], f32, tag="dy2", name="dy2")
                    nc.scalar.activation(
                        dy2[:], yj[:], AF.Square, bias=ncols[:, NT + mt : NT + mt + 1]
                    )
                    # d2 = dx2 + dy2 in place. Sample 0 builds at kernel start
                    # (GPSIMD dispatch is laggy there) -> DVE; sample 1 builds
                    # while DVE is busy with sample 0's S phase -> GPSIMD.
                    if b == 0:
                        nc.vector.tensor_add(dx2[:], dx2[:], dy2[:])
                    else:
                        nc.gpsimd.tensor_add(dx2[:], dx2[:], dy2[:])
                    nc.vector.tensor_scalar(a_row(mt), dx2[:], T2, None, OP.is_le)
                st.update(Af8=Af8, a8=a8, a_row=a_row)

            def fv0_phase(st):
                f0hl = []
                xt4 = st["xt4"]
                for mt in range(NT):
                    ms = slice(mt * 128, (mt + 1) * 128)
                    ps0 = psfv.tile([128, D], f32, tag="psf", name="ps0")
                    nc.tensor.matmul(ps0[:], xt4[:, ms], w0t[:], start=True, stop=True)
                    f0f = ptmp.tile([128, D], f32, tag="f0f", name="f0f")
                    nc.scalar.activation(f0f[:], ps0[:], AF.Relu)
                    hl = pfv.tile([128, 2 * D], bf16, tag=f"f0hl{mt}", name="f0hl")
                    nc.vector.tensor_copy(hl[:, 0:D], f0f[:])
                    nc.vector.tensor_tensor(
                        hl[:, D : 2 * D], f0f[:], hl[:, 0:D], OP.subtract
                    )
                    f0hl.append(hl)
                st["f0hl"] = f0hl

            def s_phase(st, b):
                Af8, a8 = st["Af8"], st["a8"]
                B2f8 = [
                    pb2.tile([128, 2, NP], fp8, tag=f"B2f8{s}", name=f"B2f8_{s}")
                    for s in range(NSB)
                ]
                b28 = pb2.tile([128, NP], fp8, tag="B28", name="b28")
                for mt in range(NT):
                    ms = slice(mt * 128, (mt + 1) * 128)
                    for ci, (c0, cw) in enumerate(CHUNKS):
                        ps = psbig.tile([128, 512], f32, tag="psbig", name="psS")
                        for s in range(NSB):
                            nc.tensor.matmul(
                                ps[:, 0:cw],
                                Af8[s][:, :, ms],
                                Af8[s][:, :, c0 : c0 + cw],
                                start=(s == 0),
                                stop=False,
                                perf_mode=DR,
                            )
                        nc.tensor.matmul(
                            ps[:, 0:cw],
                            a8[:, ms],
                            a8[:, c0 : c0 + cw],
                            start=False,
                            stop=True,
                        )
                        # B2 = (S > 0) for integer counts. During sample 0's S
                        # phase ACT is saturated by sample 1's squares -> DVE;
                        # during sample 1's S phase ACT is free -> ACT Sign.
                        dst = (
                            B2f8[mt // 2][:, mt % 2, c0 : c0 + cw]
                            if mt < 2 * NSB
                            else b28[:, c0 : c0 + cw]
                        )
                        if b == 0:
                            nc.vector.tensor_scalar(
                                dst, ps[:, 0:cw], 0.5, None, OP.is_ge
                            )
                        else:
                            nc.scalar.activation(dst, ps[:, 0:cw], AF.Sign)
                st.update(B2f8=B2f8, b28=b28)

            def t_phase(st):
                Af8, a8, B2f8, b28 = st["Af8"], st["a8"], st["B2f8"], st["b28"]
                CT = []
                for mt in range(NT):
                    ms = slice(mt * 128, (mt + 1) * 128)
                    ct_t = pct.tile([128, NP], bf16, tag=f"CT{mt}", name="ct")
                    if mt < 2 * NSB:
                        b2row = B2f8[mt // 2][:, mt % 2, :]
                    else:
                        b2row = b28[:]
                    for ci, (c0, cw) in enumerate(CHUNKS):
                        ps = psbig.tile([128, 512], f32, tag="psbig", name="psT")
                        for s in range(NSB):
                            nc.tensor.matmul(
                                ps[:, 0:cw],
                                Af8[s][:, :, ms],
                                B2f8[s][:, :, c0 : c0 + cw],
                                start=(s == 0),
                                stop=False,
                                perf_mode=DR,
                            )
                        nc.tensor.matmul(
                            ps[:, 0:cw],
                            a8[:, ms],
                            b28[:, c0 : c0 + cw],
                            start=False,
                            stop=True,
                        )
                        nc.vector.tensor_tensor(
                            ct_t[:, c0 : c0 + cw],
                            b2row[:, c0 : c0 + cw],
                            ps[:, 0:cw],
                            OP.mult,
                        )
                    CT.append(ct_t)
                st["CT"] = CT

            def fv1_phase(st):
                a_row, f0hl = st["a_row"], st["f0hl"]
                f1hl = []
                for mt in range(NT):
                    ms = slice(mt * 128, (mt + 1) * 128)
                    ps1 = psfv.tile([128, 2 * D], f32, tag="psf", name="ps1")
                    for kt in range(NT):
                        nc.tensor.matmul(
                            ps1[:],
                            a_row(kt)[:, ms],
                            f0hl[kt][:],
                            start=(kt == 0),
                            stop=(kt == NT - 1),
                        )
                    # fv1 = psum[:, 0:D] + psum[:, D:2D] via strided reduce
                    f1f = ptmp.tile([128, D], f32, tag="f1f", name="f1f")
                    nc.vector.tensor_reduce(
                        f1f[:],
                        ps1[:].rearrange("p (d j) -> p j d", d=2),
                        mybir.AxisListType.X,
                        OP.add,
                    )
                    hl = pfv.tile([128, 2 * D], bf16, tag=f"f1hl{mt}", name="f1hl")
                    nc.vector.tensor_copy(hl[:, 0:D], f1f[:])
                    nc.vector.tensor_tensor(
                        hl[:, D : 2 * D], f1f[:], hl[:, 0:D], OP.subtract
                    )
                    f1hl.append(hl)
                st["f1hl"] = f1hl

            def out_tile(st, b, mt):
                f2t = st["f2t"]
                ms = slice(mt * 128, (mt + 1) * 128)
                pso_t = pso.tile([128, D], f32, tag="pso", name="pso_t")
                nc.tensor.matmul(pso_t[:], f2t[:, ms], w2t[:], start=True, stop=True)
                osb = ptmp.tile([128, D], f32, tag="osb", name="osb")
                nc.vector.tensor_add(osb[:], pso_t[:], b2bc[:])
                nc.scalar.activation(osb[:], osb[:], AF.Relu)
                nc.sync.dma_start(out[b, ms, :], osb[:])

            # out row-tiles fully covered once chunk ci of f2t is written
            OUT_MTS = [range(0, 4), range(4, 8), range(8, NT)]

            def f2t_phase(st, b):
                f1hl, CT = st["f1hl"], st["CT"]
                f2t = pf2t.tile([128, NP], f32, tag="f2t", name="f2t")
                st["f2t"] = f2t
                for ci, (c0, cw) in enumerate(CHUNKS):
                    ps = psbig.tile([128, 512], f32, tag="psbig", name="psF")
                    for kt in range(NT):
                        for part in range(2):
                            nc.tensor.matmul(
                                ps[:, 0:cw],
                                f1hl[kt][:, part * D : (part + 1) * D],
                                CT[kt][:, c0 : c0 + cw],
                                start=(kt == 0 and part == 0),
                                stop=(kt == NT - 1 and part == 1),
                            )
                    nc.vector.tensor_copy(f2t[:, c0 : c0 + cw], ps[:, 0:cw])
                    for mt in OUT_MTS[ci]:
                        out_tile(st, b, mt)

            st0 = load_phase(0)
            w0t = pc.tile([4, D], f32)
            nc.sync.dma_start(w0t[:], w04[:])
            w2t = pc.tile([D, D], f32)
            nc.sync.dma_start(w2t[:], w2[:])
            b2bc = pc.tile([128, D], f32)
            nc.sync.dma_start(b2bc[:], b2r[0, :].partition_broadcast(128))
            st1 = load_phase(1)
            a_phase(st0, 0)
            fv0_phase(st0)
            fv0_phase(st1)
            a_phase(st1, 1)
            s_phase(st0, 0)
            fv1_phase(st0)
            s_phase(st1, 1)
            fv1_phase(st1)
            t_phase(st0)
            t_phase(st1)
            f2t_phase(st0, 0)
            f2t_phase(st1, 1)

    nc.compile()
    return nc


def _get_nc():
    if "nc" not in _CACHE:
        _CACHE["nc"] = _build_nc()
    return _CACHE["nc"]


def _prep_in_maps(loc, deadline, depot, W0, b0, W2, b2):
    B = loc.shape[0]
    x = np.zeros((B, NP), np.float32)
    y = np.zeros((B, NP), np.float32)
    t = np.zeros((B, NP), np.float32)
    o = np.ones((B, NP), np.float32)
    x[:, 0] = depot[:, 0]
    y[:, 0] = depot[:, 1]
    x[:, 1:N] = loc[:, :, 0]
    y[:, 1:N] = loc[:, :, 1]
    t[:, 1:N] = deadline
    # pad nodes: far away and pairwise > thresh apart
    x[:, N:] = 3.0 + np.arange(NP - N, dtype=np.float32)
    y[:, N:] = 0.5
    xq = np.stack([x, y, t, o], axis=1).astype(np.float32)       # [B, 4, NP]
    ncol = np.ascontiguousarray(
        np.stack([-x, -y], axis=1)
        .reshape(B, 2 * NT, 128)
        .transpose(0, 2, 1)
        .astype(np.float32)
    )  # [B, 128, 2*NT]: partition-major so the DMA is contiguous
    w04 = np.concatenate(
        [np.asarray(W0, np.float32), np.asarray(b0, np.float32).reshape(1, D)], axis=0
    )
    W2 = np.asarray(W2, np.float32)
    b2 = np.asarray(b2, np.float32).reshape(1, D)
    in_maps = []
    for c in range(N_CORES):
        s = slice(c * SPC, (c + 1) * SPC)
        in_maps.append(
            {
                "xq": np.ascontiguousarray(xq[s]),
                "ncol": np.ascontiguousarray(ncol[s]),
                "w04": w04,
                "w2": W2,
                "b2r": b2,
            }
        )
    return in_maps


def _run_device(in_maps):
    import time

    from concourse.bass_utils import run_bass_kernel_spmd

    nc = _get_nc()
    last_err = None
    for attempt in range(3):
        try:
            res = run_bass_kernel_spmd(nc, in_maps, list(range(N_CORES)))
            return res.results
        except Exception as e:  # transient device errors (e.g. NRT_EXEC_UNIT_*)
            last_err = e
            time.sleep(2.0 * (attempt + 1))
    raise last_err


def kernel(loc, deadline, depot, W0, b0, W2, b2):
    loc = np.asarray(loc, np.float32)
    deadline = np.asarray(deadline, np.float32)
    depot = np.asarray(depot, np.float32)
    in_maps = _prep_in_maps(loc, deadline, depot, W0, b0, W2, b2)
    results = _run_device(in_maps)
    out = np.empty((B_TOTAL, N, D), np.float32)
    for c in range(N_CORES):
        out[c * SPC : (c + 1) * SPC] = results[c]["out"][:, :N, :]
    mean = out.mean(axis=1, dtype=np.float64).astype(np.float32)
    return out, mean
